# revision 34
# baseline (speedup 1.0000x reference)
"""Trainium2 Bass kernel for nn_LocalSelfAttention (fused attention block).

Reference (B=2, S=2048, DM=1024, H=16, D=64):
  qkv = x @ Wqkv + bqkv -> split heads -> RoPE(q,k) -> softmax(q k^T/8) v
  -> concat heads @ Wo + bo -> residual + LayerNorm(gamma,beta)

Sharding (8 cores): core c = (batch c//4, query rows 512*(c%4)..+512).
K^T is projected per-core for its OWN 512 positions only and exchanged by
4-way AllGathers per batch replica group; V is recomputed redundantly.
Attention/out-proj/LN are exact and row-local; host gather is pure
concatenation.

Pipeline (v8, ~302us vs 363.8us baseline):
 * K AllGather split 4 ways (one per t-pair) with explicit input-side
   dep edges; kin staging rides the scalar queue so the sync queue
   streams the wv/xT/wq loads without head-of-line blocking.  (The
   collectives' entry barrier is a fixed ~21+30us-from-start cost; a
   dummy pre-collective cannot absorb it, so the first gathered K
   lands ~80us in regardless of trigger time.)
 * emission order K -> V -> Q -> attention: exp_end is pinned at
   (PE work before attention t=1) + 112 exps, so the V projection
   (256 MMs, kd-inner so each stationary xt slice serves both ncol
   halves) is the critical prefix.  MM issue rate measures 263ns =
   512cyc at the 13/16 power-throttled clock, LDWEIGHTS fully hidden.
 * all PSUM evacuations ride the Scalar engine while it is idle
   (before the first Exp); the 128 Exps ([128,1024] from PSUM,
   ~1.0us each) then run back-to-back and gapless.
 * rowsum reciprocal batches: heads 0-13 bounce through DRAM after
   t=6 (partition-packing keeps the iterative DVE reciprocal at
   FD=512), heads 14-15 at the tail with the bounce-back landing
   directly on partitions {0,32} (no second scatter hop).
 * score MMs carry an extra dep on the PREVIOUS kp's h0-exp so all
   four become ready together and issue adjacently: the h0/h64
   row-group pairs then stream concurrently through the PE (measured
   6ns start deltas), halving score streaming slots.
 * tail: per-t-pair normalize (two col-tiled broadcast MMs into one
   PSUM tile, Act evac, single 2x DVE multiply); LN uses bn_stats/
   bn_aggr + the Act engine for the (h-mu)*rstd affine; out-proj
   runs in four mr-quarters so each LayerNorm pipelines behind the
   next quarter's accumulation.
 * V bias folded into the residual on host (bv @ Wo term), residual
   rows shipped bf16.
"""
import numpy as np
import ml_dtypes

import concourse.bass as bass
import concourse.mybir as mybir
import concourse.tile as tile
from concourse.bass_utils import run_bass_kernel_spmd

BF16 = ml_dtypes.bfloat16
bf16 = mybir.dt.bfloat16
f32 = mybir.dt.float32
AF = mybir.ActivationFunctionType
ALU = mybir.AluOpType
AX = mybir.AxisListType

B, S, DM = 2, 2048, 1024
H, D = 16, 64
NC = 8
ROWS = S * B // NC          # 512 query rows per core
SB = S


# ---- TileContext tail-drain patch: this walrus rejects >1 sync wait on
# CTRL-class instructions; split the global-clock waits onto SP nops.
def _patched_drain_and_barrier(self, tick_clock, wait_clock):
    nc = self.nc
    drain_inst = nc.sync.drain()
    wait_clock.add_sem_waits(
        drain_inst.ins, tile.ScopedClock({None: tick_clock.global_clock})
    )
    si = drain_inst.ins.sync_info
    waits = list(si.on_wait) if si and si.on_wait else []
    if len(waits) > 1:
        si.on_wait = waits[:1]
        for w in waits[1:]:
            nop = nc.sync.nop()
            nop.ins.sync_info = mybir.SyncInfo(on_wait=[w], on_update=[])
    nc.all_engine_barrier()
    assert self.sems is not None
    popped = nc._tile_sem_poison_stack.pop()
    assert popped is self._sem_poison
    nc.all_engine_barrier()


tile.TileContext._drain_and_barrier = _patched_drain_and_barrier

_CTRL_CLASSES = ("InstNoOp", "InstDrain", "InstEventSemaphore")


def _split_excess_waits(nc, maxw_compute=1):
    """Walrus (this version) caps sync waits per instruction (1 for
    CTRL-class, ~2 for compute).  Hoist excess waits onto same-engine NoOps
    inserted immediately before the offending instruction."""
    import copy
    proto = nc.sync.nop().ins  # prototype NoOp (appended to current bb; harmless)
    proto_si = proto.sync_info
    if proto_si and proto_si.on_wait:
        proto.sync_info = mybir.SyncInfo(on_wait=[], on_update=[])
    nsplit = 0
    for f in nc.m.functions:
        for b in f.blocks:
            insts = list(b.instructions)
            out = []
            changed = False
            for inst in insts:
                cls = type(inst).__name__
                maxw = 1 if cls in _CTRL_CLASSES else maxw_compute
                si = inst.sync_info
                waits = list(si.on_wait) if si and si.on_wait else []
                if len(waits) > maxw:
                    keep = waits[:maxw]
                    extra = waits[maxw:]
                    si.on_wait = keep
                    for i, w in enumerate(extra):
                        nop = copy.deepcopy(proto)
                        nop.name = f"{inst.name}-wsplit{i}"
                        nop.engine = inst.engine
                        nop.sync_info = mybir.SyncInfo(on_wait=[w],
                                                       on_update=[])
                        out.append(nop)
                        nsplit += 1
                    changed = True
                out.append(inst)
            if changed:
                try:
                    b.instructions = out
                except Exception:
                    b.set_instructions(out)
    return nsplit


def _build_program():
    nc = bass.Bass("TRN2", target_bir_lowering=False, debug=False,
                   num_devices=NC)

    def din(name, shape, dt):
        return nc.dram_tensor(name, list(shape), dt, kind="ExternalInput").ap()

    xT = din("xT", (DM, SB), bf16)
    xTq = din("xTq", (DM, ROWS), bf16)
    xr = din("xr", (ROWS, DM), bf16)         # x rows + bo + bv@Wo (host)
    wq = din("wq", (DM, DM), bf16)
    wk = din("wk", (DM, DM), bf16)
    wv = din("wv", (DM, DM), bf16)
    wo = din("wo", (DM, DM), bf16)
    ccr = din("ccr", (128, ROWS), bf16)
    ssr = din("ssr", (128, ROWS), bf16)
    bqp = din("bqp", (128, 8), f32)
    bkp = din("bkp", (128, 8), f32)
    gbc = din("gbc", (128, DM), bf16)
    bbc = din("bbc", (128, DM), bf16)
    out = nc.dram_tensor("out", [ROWS, DM], f32, kind="ExternalOutput").ap()
    rs_dram = [nc.dram_tensor(f"rs_stage{g}", [1, (14 if g == 0 else 2) * 512],
                              bf16, kind="Internal").ap() for g in range(2)]
    rinv_dram = [nc.dram_tensor(f"rinv_stage{g}", [14 if g == 0 else 2, 512],
                                bf16, kind="Internal").ap() for g in range(2)]

    RG = [[0, 1, 2, 3], [4, 5, 6, 7]]

    with tile.TileContext(nc) as tc:
        with tc.tile_pool(name="res", bufs=1) as res, \
             tc.tile_pool(name="tmp", bufs=4) as tmp, \
             tc.tile_pool(name="ppool", bufs=6) as ppool, \
             tc.tile_pool(name="dram", bufs=1, space="DRAM") as dpool:

            xq_all = res.tile([128, 8 * ROWS], bf16, tag="xq_all")
            xq_sb = [xq_all[:, k * ROWS:(k + 1) * ROWS] for k in range(8)]
            kT = [res.tile([128, SB], bf16, name=f"kT{t}", tag=f"kT{t}") for t in range(8)]
            qT = [res.tile([128, ROWS], bf16, name=f"qT{t}", tag=f"qT{t}") for t in range(8)]
            vt = [res.tile([128, H * (D + 1)], bf16, name=f"vt{m}", tag=f"vt{m}")
                  for m in range(16)]
            aT = [res.tile([128, ROWS], bf16, name=f"aT{t}", tag=f"aT{t}") for t in range(8)]
            ccr_sb = res.tile([128, ROWS], bf16, tag="ccr")
            ssr_sb = res.tile([128, ROWS], bf16, tag="ssr")
            bq_sb = res.tile([128, 8], f32, tag="bq")
            bk_sb = res.tile([128, 8], f32, tag="bk")
            eps_sb = res.tile([128, 1], f32, tag="eps")

            # load order: K-proj inputs first (wk, xq, rope tables,
            # bias), then wq, wv, and the V-proj x^T tiles last (V MMs
            # start only after K+Q drain anyway).
            nc.sync.dma_start(
                xq_all[:].rearrange("p (a c) -> p a c", c=ROWS),
                xTq.rearrange("(a p) c -> p a c", p=128))
            nc.sync.dma_start(ccr_sb[:], ccr[:])
            nc.sync.dma_start(ssr_sb[:], ssr[:])
            nc.sync.dma_start(bk_sb[:], bkp[:])
            nc.sync.dma_start(bq_sb[:], bqp[:])
            nc.vector.memset(eps_sb[:], 1e-5)

            def rope(dst, src, cct, sst, n0, nn):
                # dst[:, n0:n0+nn] = src*CC + swap32(src)*SS
                # (cross-partition 2-input DVE ops are illegal -> copy first)
                t1 = tmp.tile([128, nn], bf16, tag="ropet1")
                t2 = tmp.tile([128, nn], bf16, tag="ropet2")
                for a, b_ in ((0, 32), (32, 0), (64, 96), (96, 64)):
                    nc.vector.tensor_copy(t2[a:a + 32, :], src[b_:b_ + 32, :])
                nc.vector.tensor_tensor(out=t1[:], in0=src[:],
                                        in1=cct[:, n0:n0 + nn], op=ALU.mult)
                nc.vector.tensor_tensor(out=t2[:], in0=t2[:],
                                        in1=sst[:, n0:n0 + nn], op=ALU.mult)
                nc.vector.tensor_tensor(out=dst[:, n0:n0 + nn], in0=t1[:],
                                        in1=t2[:], op=ALU.add)

            # ---- projections ----
            # Each core projects K only for its OWN 512 positions; four
            # 4-way AllGathers (one per head-pair tile pair, per batch
            # replica group) exchange the RoPEd K^T blocks while the PE
            # does Q and the (redundant) V projection.
            with tc.tile_pool(name="wts", bufs=1) as wts, \
                 tc.tile_pool(name="psP", bufs=4, space="PSUM") as psP:
                kin_q = [dpool.tile([256, 512], bf16, name=f"kin_{g}")
                         for g in range(4)]
                kout_q = [dpool.tile([1024, 512], bf16, name=f"kout_{g}")
                          for g in range(4)]

                wk_all = wts.tile([128, 8 * DM], bf16, tag="wk_all")
                wv_all = wts.tile([128, 8 * DM], bf16, tag="wv_all")
                xt_all = wts.tile([128, 8 * SB], bf16, tag="xt_all")
                wk_sb = [wk_all[:, k * DM:(k + 1) * DM] for k in range(8)]
                wv_sb = [wv_all[:, k * DM:(k + 1) * DM] for k in range(8)]
                xt_sb = [xt_all[:, k * SB:(k + 1) * SB] for k in range(8)]
                kT_own = [wts.tile([128, 512], bf16, name=f"ko{t}",
                                   tag=f"ko{t}") for t in range(8)]
                # one strided DMA per tensor: the sync queue pays ~290ns
                # fixed overhead per DMA instruction, which was gating the
                # V projection start (40 small loads ~ 8us of overhead).
                nc.sync.dma_start(
                    wk_all[:].rearrange("p (a c) -> p a c", c=DM),
                    wk.rearrange("(a p) c -> p a c", p=128))

                # K^T projection (own 512 positions) + RoPE, then AllGather
                cc_k = [None] * 4
                kin_dmas = []
                for t in range(8):
                    ps = psP.tile([128, 512], f32, tag="proj")
                    for kd in range(8):
                        nc.tensor.matmul(
                            ps[:], wk_sb[kd][:, t * 128:(t + 1) * 128],
                            xq_sb[kd][:], start=(kd == 0), stop=(kd == 7))
                    kt_raw = tmp.tile([128, 512], bf16, tag="evac")
                    nc.scalar.activation(kt_raw[:], ps[:], AF.Identity,
                                         bias=bk_sb[:, t:t + 1])
                    rope(kT_own[t], kt_raw, ccr_sb, ssr_sb, 0, 512)
                    g, tt = t // 2, t % 2
                    # staged from the scalar queue: a sync-queue DMA here
                    # would head-of-line-block the wq/xT/wv loads behind it
                    # while waiting on the RoPE.
                    dma = nc.scalar.dma_start(
                        kin_q[g][tt * 128:(tt + 1) * 128, :], kT_own[t][:])
                    kin_dmas.append(dma)
                    if tt == 1:
                        cc = nc.gpsimd.collective_compute(
                            "AllGather", ALU.bypass, replica_groups=RG,
                            ins=[kin_q[g].opt()], outs=[kout_q[g].opt()])
                        # DRAM tiles are not dependency-tracked: tie the
                        # trigger to the two staging DMAs explicitly.
                        for d_ in kin_dmas[-2:]:
                            bass._add_dep_helper(cc.ins, d_.ins, sync=True,
                                                 reason="AG_K input staged")
                        cc_k[g] = cc

                # wv + x^T stream right behind wk; wq last (Q-proj runs
                # AFTER the V projection, filling the PE gap between
                # V-drain and the first exp).
                nc.sync.dma_start(
                    wv_all[:].rearrange("p (a c) -> p a c", c=DM),
                    wv.rearrange("(a p) c -> p a c", p=128))
                nc.sync.dma_start(
                    xt_all[:].rearrange("p (a c) -> p a c", c=SB),
                    xT.rearrange("(a p) c -> p a c", p=128))
                wq_all = wts.tile([128, 8 * DM], bf16, tag="wk_all",
                                  name="wq_all")
                wq_sb = [wq_all[:, k * DM:(k + 1) * DM] for k in range(8)]
                nc.sync.dma_start(
                    wq_all[:].rearrange("p (a c) -> p a c", c=DM),
                    wq.rearrange("(a p) c -> p a c", p=128))

                # V projection (redundant, all 2048 positions; 65-stride
                # head slots + ones column for the softmax rowsums).
                # Scalar-engine evac: Act is idle until the first Exp.
                for m in range(16):
                    m0 = m * 128
                    pss = [psP.tile([128, 512], f32, tag="proj",
                                    name=f"vps{m}_{ncol}")
                           for ncol in range(2)]
                    # kd-inner: consecutive MM pairs share the stationary
                    # xt slice, so the weight load amortizes over 1024
                    # streamed columns.
                    for kd in range(8):
                        for ncol in range(2):
                            nc.tensor.matmul(
                                pss[ncol][:], xt_sb[kd][:, m0:m0 + 128],
                                wv_sb[kd][:, ncol * 512:ncol * 512 + 512],
                                start=(kd == 0), stop=(kd == 7))
                    for ncol in range(2):
                        dst = vt[m][:, ncol * 8 * 65:(ncol + 1) * 8 * 65]
                        dstv = dst.rearrange("p (h e) -> p h e", e=65)[:, :, 0:64]
                        srcv = pss[ncol][:].rearrange("p (h e) -> p h e", e=64)
                        nc.scalar.activation(dstv, srcv, AF.Identity)
                    onev = vt[m][:, :].rearrange("p (h e) -> p h e",
                                                 e=65)[:, :, 64:65]
                    nc.vector.memset(onev, 1.0)

                # Q^T projection + RoPE, after V: the first exp needs
                # qT[0] only once the V drain + first scores are done.
                for t in range(8):
                    ps = psP.tile([128, 512], f32, tag="proj")
                    for kd in range(8):
                        nc.tensor.matmul(
                            ps[:], wq_sb[kd][:, t * 128:(t + 1) * 128],
                            xq_sb[kd][:], start=(kd == 0), stop=(kd == 7))
                    q_raw = tmp.tile([128, ROWS], bf16, tag="evac")
                    nc.scalar.activation(q_raw[:], ps[:], AF.Identity,
                                         bias=bq_sb[:, t:t + 1])
                    rope(qT[t], q_raw, ccr_sb, ssr_sb, 0, ROWS)

                # gathered K^T -> attention layout (sync queue, after all
                # critical loads; explicit dep edges onto the collectives).
                for g in range(4):
                    for i in range(4):
                        for tt in range(2):
                            t = g * 2 + tt
                            dma = nc.sync.dma_start(
                                kT[t][:, i * 512:(i + 1) * 512],
                                kout_q[g][i * 256 + tt * 128:
                                          i * 256 + (tt + 1) * 128, :])
                            bass._add_dep_helper(dma.ins, cc_k[g].ins,
                                                 sync=True,
                                                 reason="AG_K output read")

            # ---- attention ----
            # scores transposed (S^T = K^T-chunk @ Q^T) into [128,1024] PSUM
            # mega-tiles so each Exp covers FD=1024.  The two heads of a
            # t-pair are emitted alternating so their score MMs stream
            # concurrently through PE row groups 0-63 / 64-127.  PV lags one
            # kc-pair.  Rowsums ride the V ones-column; normalization is
            # deferred and applied to the bf16 aT tiles -- heads 0-11
            # mid-attention (after t=5), heads 12-15 at the tail.
            with tc.tile_pool(name="asb", bufs=1) as asb:
                # rowsum rows stage into one SBUF row; a DRAM bounce
                # scatters them across partitions so ONE batched DVE
                # reciprocal runs at FD=512 (DVE reciprocal is iterative,
                # ~6.5ns/elem along the free dim -- partition-packing is
                # what makes it cheap).  rinvA holds the scatter back at
                # partitions {0,32}: head 2g+i -> partition 32i, col g*512.
                rs_row = asb.tile([65, H * 512], bf16, tag="rs_row")
                rsS = [asb.tile([14, 512], bf16, tag="rsS0", name="rsS0")]
                rinvS = [asb.tile([14, 512], bf16, tag="rinvS0",
                                  name="rinvS0")]
                rsSb = asb.tile([33, 512], bf16, tag="rsSb")
                rinvSb = asb.tile([33, 512], bf16, tag="rinvSb")
                rinvA = asb.tile([33, 7 * 512], bf16, tag="rinvA")
                onesA = asb.tile([33, 64], bf16, tag="onesA")
                nc.vector.memset(onesA[:], 1.0)

                with tc.tile_pool(name="psA", bufs=3, space="PSUM") as psA, \
                     tc.tile_pool(name="psO", bufs=2, space="PSUM") as psO:

                    prev_exp_h0 = [None]
                    for t in range(8):
                        oaccs = [psO.tile([65, 512], f32, tag="oacc",
                                          name=f"oacc{t}_{hh}")
                                 for hh in range(2)]
                        prev = [None, None]

                        def emit_pv(hh, kp, pT_t):
                            h = 2 * t + hh
                            for j in range(2):
                                kc = kp * 2 + j
                                nc.tensor.matmul(
                                    oaccs[hh][:],
                                    vt[kc][:, h * 65:h * 65 + 65],
                                    pT_t[:, j * 512:(j + 1) * 512],
                                    start=(kc == 0), stop=(kc == 15))

                        for kp in range(8):
                            sps = [psA.tile([128, 1024], f32, tag="sco",
                                            name=f"sco{t}_{kp}_{hh}")
                                   for hh in range(2)]
                            # hh-alternated score MMs: row groups 0-63 and
                            # 64-127 stream concurrently.
                            for j in range(2):
                                kc = kp * 2 + j
                                for hh in range(2):
                                    po = 64 * hh
                                    mm = nc.tensor.matmul(
                                        sps[hh][:, j * 512:(j + 1) * 512],
                                        kT[t][po:po + 64,
                                              kc * 128:(kc + 1) * 128],
                                        qT[t][po:po + 64, :],
                                        start=True, stop=True)
                                    # equalize readiness on the EARLIER of
                                    # the previous kp's exps: all 4 score
                                    # MMs become ready together, so the
                                    # scheduler issues them adjacently and
                                    # the h0/h64 row-group pairs can
                                    # stream concurrently.
                                    if prev_exp_h0[0] is not None:
                                        bass._add_dep_helper(
                                            mm.ins, prev_exp_h0[0].ins,
                                            sync=True,
                                            reason="score pair readiness")
                            for hh in range(2):
                                pT = ppool.tile([128, 1024], bf16, tag="pT")
                                ex = nc.scalar.activation(pT[:], sps[hh][:],
                                                          AF.Exp, scale=0.125)
                                if hh == 0:
                                    prev_exp_h0[0] = ex
                                if prev[hh] is not None:
                                    emit_pv(hh, kp - 1, prev[hh])
                                prev[hh] = pT
                        for hh in range(2):
                            emit_pv(hh, 7, prev[hh])
                        # stash rowsum rows (same-partition copies) +
                        # unnorm. attn (out-partition shift legal for
                        # 1-input copies)
                        for hh in range(2):
                            h, po = 2 * t + hh, 64 * hh
                            nc.vector.tensor_copy(
                                rs_row[64:65, h * 512:(h + 1) * 512],
                                oaccs[hh][64:65, :])
                            nc.vector.tensor_copy(aT[t][po:po + 64, :],
                                                  oaccs[hh][0:64, :])
                        # reciprocal batches: heads 0-13 after t=6 (the
                        # bounce + recip hide inside t=7's exp window);
                        # heads 14-15 at the tail via a SHORT chain: the
                        # bounce-back lands rows directly on partitions
                        # {0,32} so the broadcast MM reads the reciprocal
                        # with no second scatter hop.
                        if t == 6:
                            nc.sync.dma_start(rs_dram[0][:],
                                              rs_row[64:65, 0:14 * 512])
                            nc.sync.dma_start(
                                rsS[0][:],
                                rs_dram[0].rearrange("a (p c) -> (a p) c",
                                                     p=14))
                            with nc.allow_low_precision(
                                    reason="softmax 1/rowsum in bf16"):
                                nc.vector.reciprocal(rinvS[0][:], rsS[0][:])
                            nc.sync.dma_start(rinv_dram[0][:], rinvS[0][:])
                            for i in range(2):
                                nc.sync.dma_start(
                                    rinvA[32 * i:32 * i + 1,
                                          0:7 * 512].rearrange(
                                        "a (g c) -> a g c", c=512),
                                    rinv_dram[0].rearrange(
                                        "(g i) c -> i g c", i=2)[i:i + 1])
                        if t == 7:
                            nc.sync.dma_start(rs_dram[1][:],
                                              rs_row[64:65, 14 * 512:16 * 512])
                            for i in range(2):
                                nc.sync.dma_start(
                                    rsSb[32 * i:32 * i + 1, :],
                                    rs_dram[1][:, i * 512:(i + 1) * 512])
                            with nc.allow_low_precision(
                                    reason="softmax 1/rowsum in bf16"):
                                nc.vector.reciprocal(rinvSb[:], rsSb[:])
                # ---- out-proj + residual + LayerNorm ----
                # (psA/psO closed -> PSUM free for psF + psB.)
                # All 16 head-normalizes run here, interleaved with the
                # out-proj kd accumulation so the PE never waits: head
                # pair 2t,2t+1 normalizes, then kd=t accumulates.
                with tc.tile_pool(name="wop", bufs=1) as wop, \
                     tc.tile_pool(name="fin", bufs=2) as fin, \
                     tc.tile_pool(name="psB", bufs=3, space="PSUM") as psB, \
                     tc.tile_pool(name="psF", bufs=4, space="PSUM") as psF:

                    def normalize_pair(t):
                        # both heads' 1/rowsum broadcasts via col-tiled PE
                        # outer products into ONE PSUM tile; Act (idle
                        # post-Exp, Identity stays in the exp table set)
                        # evacuates so the DVE multiply runs in 2x bf16
                        # mode as a single [128,512] op.
                        rt, rc = (rinvA, t * 512) if t < 7 else (rinvSb, 0)
                        bc = psB.tile([128, 512], f32, tag="bc")
                        for hh in range(2):
                            po = 64 * hh
                            nc.tensor.matmul(
                                bc[po:po + 64, :],
                                onesA[32 * hh:32 * hh + 1, :],
                                rt[32 * hh:32 * hh + 1, rc:rc + 512],
                                start=True, stop=True)
                        bcs = tmp.tile([128, 512], bf16, tag="bcs")
                        nc.scalar.activation(bcs[:], bc[:], AF.Identity)
                        nc.vector.tensor_tensor(out=aT[t][:], in0=aT[t][:],
                                                in1=bcs[:], op=ALU.mult)
                    wo_sb = [wop.tile([128, DM], bf16, name=f"wo{k}",
                                      tag=f"wo{k}") for k in range(8)]
                    for k in range(8):
                        nc.sync.dma_start(wo_sb[k][:],
                                          wo[k * 128:(k + 1) * 128, :])
                    g_sb = wop.tile([128, DM], bf16, tag="g")
                    b_sb = wop.tile([128, DM], bf16, tag="b")
                    nc.sync.dma_start(g_sb[:], gbc[:])
                    nc.sync.dma_start(b_sb[:], bbc[:])

                    def outproj(psf, mrs, kds):
                        for kd in kds:
                            for mr in mrs:
                                rr = mr * 128
                                for ncol in range(2):
                                    c0 = ncol * 512
                                    nc.tensor.matmul(
                                        psf[(mr, ncol)][:],
                                        aT[kd][:, rr:rr + 128],
                                        wo_sb[kd][:, c0:c0 + 512],
                                        start=(kd == 0), stop=(kd == 7))

                    def ln_rows(psf, mr):
                        rr = mr * 128
                        xb = fin.tile([128, DM], bf16, tag="xb")
                        nc.sync.dma_start(xb[:], xr[rr:rr + 128, :])
                        # LN: one bn_stats pass for mean+var; the (h-mu)*
                        # rstd affine runs on the (post-Exp idle) Act
                        # engine as a single Identity with per-partition
                        # scale/bias.  bf16 intermediates: abs error ~2e-3
                        # of a ~5.0-max output, well inside tolerance.
                        hrow = fin.tile([128, DM], bf16, tag="hrow")
                        for ncol in range(2):
                            c0 = ncol * 512
                            nc.vector.tensor_tensor(
                                out=hrow[:, c0:c0 + 512],
                                in0=psf[(mr, ncol)][:],
                                in1=xb[:, c0:c0 + 512], op=ALU.add)
                        stats = fin.tile([128, 12], f32, tag="stats")
                        for g in range(2):
                            nc.vector.bn_stats(
                                stats[:, 6 * g:6 * g + 6],
                                hrow[:, 512 * g:512 * g + 512])
                        mv = fin.tile([128, 2], f32, tag="mv")
                        nc.vector.bn_aggr(mv[:], stats[:])
                        st = fin.tile([128, 1], f32, tag="st")
                        nc.scalar.activation(st[:], mv[:, 1:2], AF.Sqrt,
                                             bias=eps_sb[:])
                        rstd = fin.tile([128, 1], f32, tag="rstd")
                        nc.vector.reciprocal(rstd[:], st[:])
                        nmr = fin.tile([128, 1], f32, tag="nmr")
                        nc.vector.tensor_scalar(out=nmr[:], in0=mv[:, 0:1],
                                                scalar1=rstd[:],
                                                scalar2=-1.0, op0=ALU.mult,
                                                op1=ALU.mult)
                        y = fin.tile([128, DM], bf16, tag="y")
                        nc.scalar.activation(y[:], hrow[:], AF.Identity,
                                             bias=nmr[:], scale=rstd[:])
                        nc.vector.tensor_tensor(out=y[:], in0=y[:],
                                                in1=g_sb[:], op=ALU.mult)
                        yf = fin.tile([128, DM], f32, tag="yf")
                        nc.vector.tensor_tensor(out=yf[:], in0=y[:],
                                                in1=b_sb[:], op=ALU.add)
                        nc.sync.dma_start(out[rr:rr + 128, :], yf[:])

                    # 4 psF tags x 1 buf = 4 banks, two mr-rows in
                    # flight; later quarters reuse earlier slots (the
                    # LN PSUM reads are emitted first, so the slot-reuse
                    # anti-dependency edges are recorded).
                    for t in range(7):
                        normalize_pair(t)
                    psfq = []
                    for mr in range(4):
                        psf = {(mr, ncol): psF.tile(
                            [128, 512], f32, bufs=1,
                            tag=f"f{mr % 2}_{ncol}",
                            name=f"psf{mr}_{ncol}")
                            for ncol in range(2)}
                        outproj(psf, (mr,), range(7))
                        if mr == 0:
                            normalize_pair(7)
                        outproj(psf, (mr,), (7,))
                        ln_rows(psf, mr)

    _split_excess_waits(nc)
    return nc


_NC_CACHE = None


def _perm():
    p = np.zeros(DM, np.int64)
    for h in range(H):
        p[h * D:h * D + 32] = h * D + np.arange(0, D, 2)
        p[h * D + 32:(h + 1) * D] = h * D + np.arange(1, D, 2)
    return p


def kernel(x, Wqkv, bqkv, Wo, bo, gamma, beta):
    global _NC_CACHE
    x = np.asarray(x, np.float32)
    Wqkv = np.asarray(Wqkv, np.float32)
    bqkv = np.asarray(bqkv, np.float32)
    Wo = np.asarray(Wo, np.float32)
    bo = np.asarray(bo, np.float32)
    gamma = np.asarray(gamma, np.float32)
    beta = np.asarray(beta, np.float32)

    perm = _perm()
    Wq = np.ascontiguousarray(Wqkv[:, 0:DM][:, perm]).astype(BF16)
    Wk = np.ascontiguousarray(Wqkv[:, DM:2 * DM][:, perm]).astype(BF16)
    Wv = np.ascontiguousarray(Wqkv[:, 2 * DM:3 * DM]).astype(BF16)
    Wob = Wo.astype(BF16)
    bq = bqkv[0:DM][perm]
    bk = bqkv[DM:2 * DM][perm]
    bv = bqkv[2 * DM:3 * DM]

    inv = 1.0 / (10000.0 ** (np.arange(0, D, 2, dtype=np.float64) / D))
    pos = np.arange(S, dtype=np.float64)
    fr = pos[None, :] * inv[:, None]                    # [32, S]
    c32, s32 = np.cos(fr), np.sin(fr)
    CC = np.concatenate([c32, c32, c32, c32], 0).astype(BF16)   # [128, S]
    SS = np.concatenate([-s32, s32, -s32, s32], 0).astype(BF16)

    def colmajor(v):
        return np.ascontiguousarray(v.reshape(8, 128).T).astype(np.float32)

    gB = np.ascontiguousarray(np.broadcast_to(gamma, (128, DM))).astype(BF16)
    bB = np.ascontiguousarray(np.broadcast_to(beta, (128, DM))).astype(BF16)
    # bv contributes bv @ Wo to every output row (softmax weights sum to 1);
    # fold it, with bo, into the residual rows on host.
    rbias = bo + bv @ Wo

    if _NC_CACHE is None:
        _NC_CACHE = _build_program()
    nc = _NC_CACHE

    in_maps = []
    for c in range(NC):
        b, r = c // 4, c % 4
        xTb = np.ascontiguousarray(x[b].T).astype(BF16)
        rr = r * ROWS
        in_maps.append({
            "xT": xTb,
            "xTq": np.ascontiguousarray(xTb[:, rr:rr + ROWS]),
            "xr": np.ascontiguousarray(
                x[b, rr:rr + ROWS, :] + rbias[None, :]).astype(BF16),
            "wq": Wq, "wk": Wk, "wv": Wv, "wo": Wob,
            "ccr": np.ascontiguousarray(CC[:, rr:rr + ROWS]),
            "ssr": np.ascontiguousarray(SS[:, rr:rr + ROWS]),
            "bqp": colmajor(bq), "bkp": colmajor(bk),
            "gbc": gB, "bbc": bB,
        })

    res = run_bass_kernel_spmd(nc, in_maps, core_ids=list(range(NC)))
    kernel._last_results = res
    full = np.empty((B, S, DM), np.float32)
    for c in range(NC):
        b, r = c // 4, c % 4
        full[b, r * ROWS:(r + 1) * ROWS, :] = res.results[c]["out"]
    return full


# revision 35
# speedup vs baseline: 1.0138x; 1.0138x over previous
"""Trainium2 Bass kernel for nn_LocalSelfAttention (fused attention block).

Reference (B=2, S=2048, DM=1024, H=16, D=64):
  qkv = x @ Wqkv + bqkv -> split heads -> RoPE(q,k) -> softmax(q k^T/8) v
  -> concat heads @ Wo + bo -> residual + LayerNorm(gamma,beta)

Sharding (8 cores): core c = (batch c//4, query rows 512*(c%4)..+512).
K^T is projected per-core for its OWN 512 positions only and exchanged by
4-way AllGathers per batch replica group; V is recomputed redundantly.
Attention/out-proj/LN are exact and row-local; host gather is pure
concatenation.

Pipeline (v8, ~302us vs 363.8us baseline):
 * K AllGather split 4 ways (one per t-pair) with explicit input-side
   dep edges; kin staging rides the scalar queue so the sync queue
   streams the wv/xT/wq loads without head-of-line blocking.  (The
   collectives' entry barrier is a fixed ~21+30us-from-start cost; a
   dummy pre-collective cannot absorb it, so the first gathered K
   lands ~80us in regardless of trigger time.)
 * emission order K -> V -> Q -> attention: exp_end is pinned at
   (PE work before attention t=1) + 112 exps, so the V projection
   (256 MMs, kd-inner so each stationary xt slice serves both ncol
   halves) is the critical prefix.  MM issue rate measures 263ns =
   512cyc at the 13/16 power-throttled clock, LDWEIGHTS fully hidden.
 * all PSUM evacuations ride the Scalar engine while it is idle
   (before the first Exp); the 128 Exps ([128,1024] from PSUM,
   ~1.0us each) then run back-to-back and gapless.
 * rowsum reciprocal batches: heads 0-13 bounce through DRAM after
   t=6 (partition-packing keeps the iterative DVE reciprocal at
   FD=512), heads 14-15 at the tail with the bounce-back landing
   directly on partitions {0,32} (no second scatter hop).
 * score MMs carry an extra dep on the PREVIOUS kp's h0-exp so all
   four become ready together and issue adjacently: the h0/h64
   row-group pairs then stream concurrently through the PE (measured
   6ns start deltas), halving score streaming slots.
 * tail: per-t-pair normalize (two col-tiled broadcast MMs into one
   PSUM tile, Act evac, single 2x DVE multiply); LN uses bn_stats/
   bn_aggr + the Act engine for the (h-mu)*rstd affine; out-proj
   runs in four mr-quarters so each LayerNorm pipelines behind the
   next quarter's accumulation.
 * V bias folded into the residual on host (bv @ Wo term), residual
   rows shipped bf16.
"""
import numpy as np
import ml_dtypes

import concourse.bass as bass
import concourse.mybir as mybir
import concourse.tile as tile
from concourse.bass_utils import run_bass_kernel_spmd

BF16 = ml_dtypes.bfloat16
bf16 = mybir.dt.bfloat16
f32 = mybir.dt.float32
AF = mybir.ActivationFunctionType
ALU = mybir.AluOpType
AX = mybir.AxisListType

B, S, DM = 2, 2048, 1024
H, D = 16, 64
NC = 8
ROWS = S * B // NC          # 512 query rows per core
SB = S


# ---- TileContext tail-drain patch: this walrus rejects >1 sync wait on
# CTRL-class instructions; split the global-clock waits onto SP nops.
def _patched_drain_and_barrier(self, tick_clock, wait_clock):
    nc = self.nc
    drain_inst = nc.sync.drain()
    wait_clock.add_sem_waits(
        drain_inst.ins, tile.ScopedClock({None: tick_clock.global_clock})
    )
    si = drain_inst.ins.sync_info
    waits = list(si.on_wait) if si and si.on_wait else []
    if len(waits) > 1:
        si.on_wait = waits[:1]
        for w in waits[1:]:
            nop = nc.sync.nop()
            nop.ins.sync_info = mybir.SyncInfo(on_wait=[w], on_update=[])
    nc.all_engine_barrier()
    assert self.sems is not None
    popped = nc._tile_sem_poison_stack.pop()
    assert popped is self._sem_poison
    nc.all_engine_barrier()


tile.TileContext._drain_and_barrier = _patched_drain_and_barrier

_CTRL_CLASSES = ("InstNoOp", "InstDrain", "InstEventSemaphore")


def _split_excess_waits(nc, maxw_compute=1):
    """Walrus (this version) caps sync waits per instruction (1 for
    CTRL-class, ~2 for compute).  Hoist excess waits onto same-engine NoOps
    inserted immediately before the offending instruction."""
    import copy
    proto = nc.sync.nop().ins  # prototype NoOp (appended to current bb; harmless)
    proto_si = proto.sync_info
    if proto_si and proto_si.on_wait:
        proto.sync_info = mybir.SyncInfo(on_wait=[], on_update=[])
    nsplit = 0
    for f in nc.m.functions:
        for b in f.blocks:
            insts = list(b.instructions)
            out = []
            changed = False
            for inst in insts:
                cls = type(inst).__name__
                maxw = 1 if cls in _CTRL_CLASSES else maxw_compute
                si = inst.sync_info
                waits = list(si.on_wait) if si and si.on_wait else []
                if len(waits) > maxw:
                    keep = waits[:maxw]
                    extra = waits[maxw:]
                    si.on_wait = keep
                    for i, w in enumerate(extra):
                        nop = copy.deepcopy(proto)
                        nop.name = f"{inst.name}-wsplit{i}"
                        nop.engine = inst.engine
                        nop.sync_info = mybir.SyncInfo(on_wait=[w],
                                                       on_update=[])
                        out.append(nop)
                        nsplit += 1
                    changed = True
                out.append(inst)
            if changed:
                try:
                    b.instructions = out
                except Exception:
                    b.set_instructions(out)
    return nsplit


def _build_program():
    nc = bass.Bass("TRN2", target_bir_lowering=False, debug=False,
                   num_devices=NC)

    def din(name, shape, dt):
        return nc.dram_tensor(name, list(shape), dt, kind="ExternalInput").ap()

    xT = din("xT", (DM, SB), bf16)
    xTq = din("xTq", (DM, ROWS), bf16)
    xr = din("xr", (ROWS, DM), bf16)         # x rows + bo + bv@Wo (host)
    wq = din("wq", (DM, DM), bf16)
    wk = din("wk", (DM, DM), bf16)
    wv = din("wv", (DM, DM), bf16)
    wo = din("wo", (DM, DM), bf16)
    ccr = din("ccr", (128, ROWS), bf16)
    ssr = din("ssr", (128, ROWS), bf16)
    bqp = din("bqp", (128, 8), f32)
    bkp = din("bkp", (128, 8), f32)
    gbc = din("gbc", (128, DM), bf16)
    bbc = din("bbc", (128, DM), bf16)
    out = nc.dram_tensor("out", [ROWS, DM], f32, kind="ExternalOutput").ap()
    rs_dram = [nc.dram_tensor(f"rs_stage{g}", [1, (14 if g == 0 else 2) * 512],
                              bf16, kind="Internal").ap() for g in range(2)]
    rinv_dram = [nc.dram_tensor(f"rinv_stage{g}", [14 if g == 0 else 2, 512],
                                bf16, kind="Internal").ap() for g in range(2)]

    RG = [[0, 1, 2, 3], [4, 5, 6, 7]]

    with tile.TileContext(nc) as tc:
        with tc.tile_pool(name="res", bufs=1) as res, \
             tc.tile_pool(name="tmp", bufs=4) as tmp, \
             tc.tile_pool(name="ppool", bufs=6) as ppool, \
             tc.tile_pool(name="dram", bufs=1, space="DRAM") as dpool:

            xq_sb = [res.tile([128, ROWS], bf16, name=f"xq{k}", tag=f"xq{k}") for k in range(8)]
            kT = [res.tile([128, SB], bf16, name=f"kT{t}", tag=f"kT{t}") for t in range(8)]
            qT = [res.tile([128, ROWS], bf16, name=f"qT{t}", tag=f"qT{t}") for t in range(8)]
            vt = [res.tile([128, H * (D + 1)], bf16, name=f"vt{m}", tag=f"vt{m}")
                  for m in range(16)]
            aT = [res.tile([128, ROWS], bf16, name=f"aT{t}", tag=f"aT{t}") for t in range(8)]
            ccr_sb = res.tile([128, ROWS], bf16, tag="ccr")
            ssr_sb = res.tile([128, ROWS], bf16, tag="ssr")
            bq_sb = res.tile([128, 8], f32, tag="bq")
            bk_sb = res.tile([128, 8], f32, tag="bk")
            eps_sb = res.tile([128, 1], f32, tag="eps")

            # load order: K-proj inputs first (wk, xq, rope tables,
            # bias), then wq, wv, and the V-proj x^T tiles last (V MMs
            # start only after K+Q drain anyway).
            for k in range(8):
                nc.sync.dma_start(xq_sb[k][:], xTq[k * 128:(k + 1) * 128, :])
            nc.sync.dma_start(ccr_sb[:], ccr[:])
            nc.sync.dma_start(ssr_sb[:], ssr[:])
            nc.sync.dma_start(bk_sb[:], bkp[:])
            nc.sync.dma_start(bq_sb[:], bqp[:])
            nc.vector.memset(eps_sb[:], 1e-5)

            def rope(dst, src, cct, sst, n0, nn):
                # dst[:, n0:n0+nn] = src*CC + swap32(src)*SS
                # (cross-partition 2-input DVE ops are illegal -> copy first)
                t1 = tmp.tile([128, nn], bf16, tag="ropet1")
                t2 = tmp.tile([128, nn], bf16, tag="ropet2")
                for a, b_ in ((0, 32), (32, 0), (64, 96), (96, 64)):
                    nc.vector.tensor_copy(t2[a:a + 32, :], src[b_:b_ + 32, :])
                nc.vector.tensor_tensor(out=t1[:], in0=src[:],
                                        in1=cct[:, n0:n0 + nn], op=ALU.mult)
                nc.vector.tensor_tensor(out=t2[:], in0=t2[:],
                                        in1=sst[:, n0:n0 + nn], op=ALU.mult)
                nc.vector.tensor_tensor(out=dst[:, n0:n0 + nn], in0=t1[:],
                                        in1=t2[:], op=ALU.add)

            # ---- projections ----
            # Each core projects K only for its OWN 512 positions; four
            # 4-way AllGathers (one per head-pair tile pair, per batch
            # replica group) exchange the RoPEd K^T blocks while the PE
            # does Q and the (redundant) V projection.
            with tc.tile_pool(name="wts", bufs=1) as wts, \
                 tc.tile_pool(name="psP", bufs=4, space="PSUM") as psP:
                kin_q = [dpool.tile([256, 512], bf16, name=f"kin_{g}")
                         for g in range(4)]
                kout_q = [dpool.tile([1024, 512], bf16, name=f"kout_{g}")
                          for g in range(4)]

                wk_sb = [wts.tile([128, DM], bf16, name=f"wk{k}", tag=f"wk{k}")
                         for k in range(8)]
                wv_sb = [wts.tile([128, DM], bf16, name=f"wv{k}", tag=f"wv{k}")
                         for k in range(8)]
                xt_sb = [wts.tile([128, SB], bf16, name=f"xt{k}",
                                  tag=f"xt{k}") for k in range(8)]
                kT_own = [wts.tile([128, 512], bf16, name=f"ko{t}",
                                   tag=f"ko{t}") for t in range(8)]
                for k in range(8):
                    nc.sync.dma_start(wk_sb[k][:], wk[k * 128:(k + 1) * 128, :])

                # K^T projection (own 512 positions) + RoPE, then AllGather
                cc_k = [None] * 4
                kin_dmas = []
                for t in range(8):
                    ps = psP.tile([128, 512], f32, tag="proj")
                    for kd in range(8):
                        nc.tensor.matmul(
                            ps[:], wk_sb[kd][:, t * 128:(t + 1) * 128],
                            xq_sb[kd][:], start=(kd == 0), stop=(kd == 7))
                    kt_raw = tmp.tile([128, 512], bf16, tag="evac")
                    nc.scalar.activation(kt_raw[:], ps[:], AF.Identity,
                                         bias=bk_sb[:, t:t + 1])
                    rope(kT_own[t], kt_raw, ccr_sb, ssr_sb, 0, 512)
                    g, tt = t // 2, t % 2
                    # staged from the scalar queue: a sync-queue DMA here
                    # would head-of-line-block the wq/xT/wv loads behind it
                    # while waiting on the RoPE.
                    dma = nc.scalar.dma_start(
                        kin_q[g][tt * 128:(tt + 1) * 128, :], kT_own[t][:])
                    kin_dmas.append(dma)
                    if tt == 1:
                        cc = nc.gpsimd.collective_compute(
                            "AllGather", ALU.bypass, replica_groups=RG,
                            ins=[kin_q[g].opt()], outs=[kout_q[g].opt()])
                        # DRAM tiles are not dependency-tracked: tie the
                        # trigger to the two staging DMAs explicitly.
                        for d_ in kin_dmas[-2:]:
                            bass._add_dep_helper(cc.ins, d_.ins, sync=True,
                                                 reason="AG_K input staged")
                        cc_k[g] = cc

                # wv + x^T stream right behind wk; wq last (Q-proj runs
                # AFTER the V projection, filling the PE gap between
                # V-drain and the first exp).
                for k in range(8):
                    nc.sync.dma_start(wv_sb[k][:], wv[k * 128:(k + 1) * 128, :])
                for k in range(8):
                    nc.sync.dma_start(xt_sb[k][:], xT[k * 128:(k + 1) * 128, :])
                wq_sb = [wts.tile([128, DM], bf16, name=f"wq{k}", tag=f"wk{k}")
                         for k in range(8)]
                for k in range(8):
                    nc.sync.dma_start(wq_sb[k][:], wq[k * 128:(k + 1) * 128, :])

                # V projection (redundant, all 2048 positions; 65-stride
                # head slots + ones column for the softmax rowsums).
                # Scalar-engine evac: Act is idle until the first Exp.
                for m in range(16):
                    m0 = m * 128
                    pss = [psP.tile([128, 512], f32, tag="proj",
                                    name=f"vps{m}_{ncol}")
                           for ncol in range(2)]
                    # kd-inner: consecutive MM pairs share the stationary
                    # xt slice, so the weight load amortizes over 1024
                    # streamed columns.
                    for kd in range(8):
                        for ncol in range(2):
                            nc.tensor.matmul(
                                pss[ncol][:], xt_sb[kd][:, m0:m0 + 128],
                                wv_sb[kd][:, ncol * 512:ncol * 512 + 512],
                                start=(kd == 0), stop=(kd == 7))
                    for ncol in range(2):
                        dst = vt[m][:, ncol * 8 * 65:(ncol + 1) * 8 * 65]
                        dstv = dst.rearrange("p (h e) -> p h e", e=65)[:, :, 0:64]
                        srcv = pss[ncol][:].rearrange("p (h e) -> p h e", e=64)
                        nc.scalar.activation(dstv, srcv, AF.Identity)
                    onev = vt[m][:, :].rearrange("p (h e) -> p h e",
                                                 e=65)[:, :, 64:65]
                    nc.vector.memset(onev, 1.0)

                # Q^T projection + RoPE, after V: the first exp needs
                # qT[0] only once the V drain + first scores are done.
                for t in range(8):
                    ps = psP.tile([128, 512], f32, tag="proj")
                    for kd in range(8):
                        nc.tensor.matmul(
                            ps[:], wq_sb[kd][:, t * 128:(t + 1) * 128],
                            xq_sb[kd][:], start=(kd == 0), stop=(kd == 7))
                    q_raw = tmp.tile([128, ROWS], bf16, tag="evac")
                    nc.scalar.activation(q_raw[:], ps[:], AF.Identity,
                                         bias=bq_sb[:, t:t + 1])
                    rope(qT[t], q_raw, ccr_sb, ssr_sb, 0, ROWS)

                # gathered K^T -> attention layout (sync queue, after all
                # critical loads; explicit dep edges onto the collectives).
                for g in range(4):
                    for i in range(4):
                        for tt in range(2):
                            t = g * 2 + tt
                            dma = nc.sync.dma_start(
                                kT[t][:, i * 512:(i + 1) * 512],
                                kout_q[g][i * 256 + tt * 128:
                                          i * 256 + (tt + 1) * 128, :])
                            bass._add_dep_helper(dma.ins, cc_k[g].ins,
                                                 sync=True,
                                                 reason="AG_K output read")

            # ---- attention ----
            # scores transposed (S^T = K^T-chunk @ Q^T) into [128,1024] PSUM
            # mega-tiles so each Exp covers FD=1024.  The two heads of a
            # t-pair are emitted alternating so their score MMs stream
            # concurrently through PE row groups 0-63 / 64-127.  PV lags one
            # kc-pair.  Rowsums ride the V ones-column; normalization is
            # deferred and applied to the bf16 aT tiles -- heads 0-11
            # mid-attention (after t=5), heads 12-15 at the tail.
            with tc.tile_pool(name="asb", bufs=1) as asb:
                # rowsum rows stage into one SBUF row; a DRAM bounce
                # scatters them across partitions so ONE batched DVE
                # reciprocal runs at FD=512 (DVE reciprocal is iterative,
                # ~6.5ns/elem along the free dim -- partition-packing is
                # what makes it cheap).  rinvA holds the scatter back at
                # partitions {0,32}: head 2g+i -> partition 32i, col g*512.
                rs_row = asb.tile([65, H * 512], bf16, tag="rs_row")
                rsS = [asb.tile([14, 512], bf16, tag="rsS0", name="rsS0")]
                rinvS = [asb.tile([14, 512], bf16, tag="rinvS0",
                                  name="rinvS0")]
                rsSb = asb.tile([33, 512], bf16, tag="rsSb")
                rinvSb = asb.tile([33, 512], bf16, tag="rinvSb")
                rinvA = asb.tile([33, 7 * 512], bf16, tag="rinvA")
                onesA = asb.tile([33, 64], bf16, tag="onesA")
                nc.vector.memset(onesA[:], 1.0)

                with tc.tile_pool(name="psA", bufs=3, space="PSUM") as psA, \
                     tc.tile_pool(name="psO", bufs=2, space="PSUM") as psO:

                    prev_exp_h0 = [None]
                    for t in range(8):
                        oaccs = [psO.tile([65, 512], f32, tag="oacc",
                                          name=f"oacc{t}_{hh}")
                                 for hh in range(2)]
                        prev = [None, None]

                        def emit_pv(hh, kp, pT_t):
                            h = 2 * t + hh
                            for j in range(2):
                                kc = kp * 2 + j
                                nc.tensor.matmul(
                                    oaccs[hh][:],
                                    vt[kc][:, h * 65:h * 65 + 65],
                                    pT_t[:, j * 512:(j + 1) * 512],
                                    start=(kc == 0), stop=(kc == 15))

                        for kp in range(8):
                            sps = [psA.tile([128, 1024], f32, tag="sco",
                                            name=f"sco{t}_{kp}_{hh}")
                                   for hh in range(2)]
                            # hh-alternated score MMs: row groups 0-63 and
                            # 64-127 stream concurrently.
                            for j in range(2):
                                kc = kp * 2 + j
                                for hh in range(2):
                                    po = 64 * hh
                                    mm = nc.tensor.matmul(
                                        sps[hh][:, j * 512:(j + 1) * 512],
                                        kT[t][po:po + 64,
                                              kc * 128:(kc + 1) * 128],
                                        qT[t][po:po + 64, :],
                                        start=True, stop=True)
                                    # equalize readiness on the EARLIER of
                                    # the previous kp's exps: all 4 score
                                    # MMs become ready together, so the
                                    # scheduler issues them adjacently and
                                    # the h0/h64 row-group pairs can
                                    # stream concurrently.
                                    if prev_exp_h0[0] is not None:
                                        bass._add_dep_helper(
                                            mm.ins, prev_exp_h0[0].ins,
                                            sync=True,
                                            reason="score pair readiness")
                            for hh in range(2):
                                pT = ppool.tile([128, 1024], bf16, tag="pT")
                                ex = nc.scalar.activation(pT[:], sps[hh][:],
                                                          AF.Exp, scale=0.125)
                                if hh == 0:
                                    prev_exp_h0[0] = ex
                                if prev[hh] is not None:
                                    emit_pv(hh, kp - 1, prev[hh])
                                prev[hh] = pT
                        for hh in range(2):
                            emit_pv(hh, 7, prev[hh])
                        # stash rowsum rows (same-partition copies) +
                        # unnorm. attn (out-partition shift legal for
                        # 1-input copies)
                        for hh in range(2):
                            h, po = 2 * t + hh, 64 * hh
                            nc.vector.tensor_copy(
                                rs_row[64:65, h * 512:(h + 1) * 512],
                                oaccs[hh][64:65, :])
                            nc.vector.tensor_copy(aT[t][po:po + 64, :],
                                                  oaccs[hh][0:64, :])
                        # reciprocal batches: heads 0-13 after t=6 (the
                        # bounce + recip hide inside t=7's exp window);
                        # heads 14-15 at the tail via a SHORT chain: the
                        # bounce-back lands rows directly on partitions
                        # {0,32} so the broadcast MM reads the reciprocal
                        # with no second scatter hop.
                        if t == 6:
                            nc.sync.dma_start(rs_dram[0][:],
                                              rs_row[64:65, 0:14 * 512])
                            nc.sync.dma_start(
                                rsS[0][:],
                                rs_dram[0].rearrange("a (p c) -> (a p) c",
                                                     p=14))
                            with nc.allow_low_precision(
                                    reason="softmax 1/rowsum in bf16"):
                                nc.vector.reciprocal(rinvS[0][:], rsS[0][:])
                            nc.sync.dma_start(rinv_dram[0][:], rinvS[0][:])
                            for i in range(2):
                                nc.sync.dma_start(
                                    rinvA[32 * i:32 * i + 1,
                                          0:7 * 512].rearrange(
                                        "a (g c) -> a g c", c=512),
                                    rinv_dram[0].rearrange(
                                        "(g i) c -> i g c", i=2)[i:i + 1])
                        if t == 7:
                            nc.sync.dma_start(rs_dram[1][:],
                                              rs_row[64:65, 14 * 512:16 * 512])
                            for i in range(2):
                                nc.sync.dma_start(
                                    rsSb[32 * i:32 * i + 1, :],
                                    rs_dram[1][:, i * 512:(i + 1) * 512])
                            with nc.allow_low_precision(
                                    reason="softmax 1/rowsum in bf16"):
                                nc.vector.reciprocal(rinvSb[:], rsSb[:])
                # ---- out-proj + residual + LayerNorm ----
                # (psA/psO closed -> PSUM free for psF + psB.)
                # All 16 head-normalizes run here, interleaved with the
                # out-proj kd accumulation so the PE never waits: head
                # pair 2t,2t+1 normalizes, then kd=t accumulates.
                with tc.tile_pool(name="wop", bufs=1) as wop, \
                     tc.tile_pool(name="fin", bufs=2) as fin, \
                     tc.tile_pool(name="psB", bufs=3, space="PSUM") as psB, \
                     tc.tile_pool(name="psF", bufs=4, space="PSUM") as psF:

                    def normalize_pair(t):
                        # both heads' 1/rowsum broadcasts via col-tiled PE
                        # outer products into ONE PSUM tile; Act (idle
                        # post-Exp, Identity stays in the exp table set)
                        # evacuates so the DVE multiply runs in 2x bf16
                        # mode as a single [128,512] op.
                        rt, rc = (rinvA, t * 512) if t < 7 else (rinvSb, 0)
                        bc = psB.tile([128, 512], f32, tag="bc")
                        for hh in range(2):
                            po = 64 * hh
                            nc.tensor.matmul(
                                bc[po:po + 64, :],
                                onesA[32 * hh:32 * hh + 1, :],
                                rt[32 * hh:32 * hh + 1, rc:rc + 512],
                                start=True, stop=True)
                        bcs = tmp.tile([128, 512], bf16, tag="bcs")
                        nc.scalar.activation(bcs[:], bc[:], AF.Identity)
                        nc.vector.tensor_tensor(out=aT[t][:], in0=aT[t][:],
                                                in1=bcs[:], op=ALU.mult)
                    wo_sb = [wop.tile([128, DM], bf16, name=f"wo{k}",
                                      tag=f"wo{k}") for k in range(8)]
                    for k in range(8):
                        nc.sync.dma_start(wo_sb[k][:],
                                          wo[k * 128:(k + 1) * 128, :])
                    g_sb = wop.tile([128, DM], bf16, tag="g")
                    b_sb = wop.tile([128, DM], bf16, tag="b")
                    nc.sync.dma_start(g_sb[:], gbc[:])
                    nc.sync.dma_start(b_sb[:], bbc[:])

                    def outproj(psf, mrs, kds):
                        for kd in kds:
                            for mr in mrs:
                                rr = mr * 128
                                for ncol in range(2):
                                    c0 = ncol * 512
                                    nc.tensor.matmul(
                                        psf[(mr, ncol)][:],
                                        aT[kd][:, rr:rr + 128],
                                        wo_sb[kd][:, c0:c0 + 512],
                                        start=(kd == 0), stop=(kd == 7))

                    def ln_rows(psf, mr):
                        rr = mr * 128
                        xb = fin.tile([128, DM], bf16, tag="xb")
                        nc.sync.dma_start(xb[:], xr[rr:rr + 128, :])
                        # LN: one bn_stats pass for mean+var; the (h-mu)*
                        # rstd affine runs on the (post-Exp idle) Act
                        # engine as a single Identity with per-partition
                        # scale/bias.  bf16 intermediates: abs error ~2e-3
                        # of a ~5.0-max output, well inside tolerance.
                        hrow = fin.tile([128, DM], bf16, tag="hrow")
                        for ncol in range(2):
                            c0 = ncol * 512
                            nc.vector.tensor_tensor(
                                out=hrow[:, c0:c0 + 512],
                                in0=psf[(mr, ncol)][:],
                                in1=xb[:, c0:c0 + 512], op=ALU.add)
                        stats = fin.tile([128, 12], f32, tag="stats")
                        for g in range(2):
                            nc.vector.bn_stats(
                                stats[:, 6 * g:6 * g + 6],
                                hrow[:, 512 * g:512 * g + 512])
                        mv = fin.tile([128, 2], f32, tag="mv")
                        nc.vector.bn_aggr(mv[:], stats[:])
                        st = fin.tile([128, 1], f32, tag="st")
                        nc.scalar.activation(st[:], mv[:, 1:2], AF.Sqrt,
                                             bias=eps_sb[:])
                        rstd = fin.tile([128, 1], f32, tag="rstd")
                        nc.vector.reciprocal(rstd[:], st[:])
                        nmr = fin.tile([128, 1], f32, tag="nmr")
                        nc.vector.tensor_scalar(out=nmr[:], in0=mv[:, 0:1],
                                                scalar1=rstd[:],
                                                scalar2=-1.0, op0=ALU.mult,
                                                op1=ALU.mult)
                        y = fin.tile([128, DM], bf16, tag="y")
                        nc.scalar.activation(y[:], hrow[:], AF.Identity,
                                             bias=nmr[:], scale=rstd[:])
                        nc.vector.tensor_tensor(out=y[:], in0=y[:],
                                                in1=g_sb[:], op=ALU.mult)
                        yf = fin.tile([128, DM], f32, tag="yf")
                        nc.vector.tensor_tensor(out=yf[:], in0=y[:],
                                                in1=b_sb[:], op=ALU.add)
                        nc.sync.dma_start(out[rr:rr + 128, :], yf[:])

                    # 4 psF tags x 1 buf = 4 banks, two mr-rows in
                    # flight; later quarters reuse earlier slots (the
                    # LN PSUM reads are emitted first, so the slot-reuse
                    # anti-dependency edges are recorded).
                    for t in range(7):
                        normalize_pair(t)
                    psfq = []
                    for mr in range(4):
                        psf = {(mr, ncol): psF.tile(
                            [128, 512], f32, bufs=1,
                            tag=f"f{mr % 2}_{ncol}",
                            name=f"psf{mr}_{ncol}")
                            for ncol in range(2)}
                        outproj(psf, (mr,), range(7))
                        if mr == 0:
                            normalize_pair(7)
                        outproj(psf, (mr,), (7,))
                        ln_rows(psf, mr)

    _split_excess_waits(nc)
    return nc


_NC_CACHE = None


def _perm():
    p = np.zeros(DM, np.int64)
    for h in range(H):
        p[h * D:h * D + 32] = h * D + np.arange(0, D, 2)
        p[h * D + 32:(h + 1) * D] = h * D + np.arange(1, D, 2)
    return p


def kernel(x, Wqkv, bqkv, Wo, bo, gamma, beta):
    global _NC_CACHE
    x = np.asarray(x, np.float32)
    Wqkv = np.asarray(Wqkv, np.float32)
    bqkv = np.asarray(bqkv, np.float32)
    Wo = np.asarray(Wo, np.float32)
    bo = np.asarray(bo, np.float32)
    gamma = np.asarray(gamma, np.float32)
    beta = np.asarray(beta, np.float32)

    perm = _perm()
    Wq = np.ascontiguousarray(Wqkv[:, 0:DM][:, perm]).astype(BF16)
    Wk = np.ascontiguousarray(Wqkv[:, DM:2 * DM][:, perm]).astype(BF16)
    Wv = np.ascontiguousarray(Wqkv[:, 2 * DM:3 * DM]).astype(BF16)
    Wob = Wo.astype(BF16)
    bq = bqkv[0:DM][perm]
    bk = bqkv[DM:2 * DM][perm]
    bv = bqkv[2 * DM:3 * DM]

    inv = 1.0 / (10000.0 ** (np.arange(0, D, 2, dtype=np.float64) / D))
    pos = np.arange(S, dtype=np.float64)
    fr = pos[None, :] * inv[:, None]                    # [32, S]
    c32, s32 = np.cos(fr), np.sin(fr)
    CC = np.concatenate([c32, c32, c32, c32], 0).astype(BF16)   # [128, S]
    SS = np.concatenate([-s32, s32, -s32, s32], 0).astype(BF16)

    def colmajor(v):
        return np.ascontiguousarray(v.reshape(8, 128).T).astype(np.float32)

    gB = np.ascontiguousarray(np.broadcast_to(gamma, (128, DM))).astype(BF16)
    bB = np.ascontiguousarray(np.broadcast_to(beta, (128, DM))).astype(BF16)
    # bv contributes bv @ Wo to every output row (softmax weights sum to 1);
    # fold it, with bo, into the residual rows on host.
    rbias = bo + bv @ Wo

    if _NC_CACHE is None:
        _NC_CACHE = _build_program()
    nc = _NC_CACHE

    in_maps = []
    for c in range(NC):
        b, r = c // 4, c % 4
        xTb = np.ascontiguousarray(x[b].T).astype(BF16)
        rr = r * ROWS
        in_maps.append({
            "xT": xTb,
            "xTq": np.ascontiguousarray(xTb[:, rr:rr + ROWS]),
            "xr": np.ascontiguousarray(
                x[b, rr:rr + ROWS, :] + rbias[None, :]).astype(BF16),
            "wq": Wq, "wk": Wk, "wv": Wv, "wo": Wob,
            "ccr": np.ascontiguousarray(CC[:, rr:rr + ROWS]),
            "ssr": np.ascontiguousarray(SS[:, rr:rr + ROWS]),
            "bqp": colmajor(bq), "bkp": colmajor(bk),
            "gbc": gB, "bbc": bB,
        })

    res = run_bass_kernel_spmd(nc, in_maps, core_ids=list(range(NC)))
    kernel._last_results = res
    full = np.empty((B, S, DM), np.float32)
    for c in range(NC):
        b, r = c // 4, c % 4
        full[b, r * ROWS:(r + 1) * ROWS, :] = res.results[c]["out"]
    return full


# revision 36
# speedup vs baseline: 1.0190x; 1.0051x over previous
"""Trainium2 Bass kernel for nn_LocalSelfAttention (fused attention block).

Reference (B=2, S=2048, DM=1024, H=16, D=64):
  qkv = x @ Wqkv + bqkv -> split heads -> RoPE(q,k) -> softmax(q k^T/8) v
  -> concat heads @ Wo + bo -> residual + LayerNorm(gamma,beta)

Sharding (8 cores): core c = (batch c//4, query rows 512*(c%4)..+512).
K^T is projected per-core for its OWN 512 positions only and exchanged by
4-way AllGathers per batch replica group; V is recomputed redundantly.
Attention/out-proj/LN are exact and row-local; host gather is pure
concatenation.

Pipeline (v8, ~302us vs 363.8us baseline):
 * K AllGather split 4 ways (one per t-pair) with explicit input-side
   dep edges; kin staging rides the scalar queue so the sync queue
   streams the wv/xT/wq loads without head-of-line blocking.  (The
   collectives' entry barrier is a fixed ~21+30us-from-start cost; a
   dummy pre-collective cannot absorb it, so the first gathered K
   lands ~80us in regardless of trigger time.)
 * emission order K -> V -> Q -> attention: exp_end is pinned at
   (PE work before attention t=1) + 112 exps, so the V projection
   (256 MMs, kd-inner so each stationary xt slice serves both ncol
   halves) is the critical prefix.  MM issue rate measures 263ns =
   512cyc at the 13/16 power-throttled clock, LDWEIGHTS fully hidden.
 * all PSUM evacuations ride the Scalar engine while it is idle
   (before the first Exp); the 128 Exps ([128,1024] from PSUM,
   ~1.0us each) then run back-to-back and gapless.
 * rowsum reciprocal batches: heads 0-13 bounce through DRAM after
   t=6 (partition-packing keeps the iterative DVE reciprocal at
   FD=512), heads 14-15 at the tail with the bounce-back landing
   directly on partitions {0,32} (no second scatter hop).
 * score MMs carry an extra dep on the PREVIOUS kp's h0-exp so all
   four become ready together and issue adjacently: the h0/h64
   row-group pairs then stream concurrently through the PE (measured
   6ns start deltas), halving score streaming slots.
 * tail: per-t-pair normalize (two col-tiled broadcast MMs into one
   PSUM tile, Act evac, single 2x DVE multiply); LN uses bn_stats/
   bn_aggr + the Act engine for the (h-mu)*rstd affine; out-proj
   runs in four mr-quarters so each LayerNorm pipelines behind the
   next quarter's accumulation.
 * V bias folded into the residual on host (bv @ Wo term), residual
   rows shipped bf16.
"""
import numpy as np
import ml_dtypes

import concourse.bass as bass
import concourse.mybir as mybir
import concourse.tile as tile
from concourse.bass_utils import run_bass_kernel_spmd

BF16 = ml_dtypes.bfloat16
bf16 = mybir.dt.bfloat16
f32 = mybir.dt.float32
AF = mybir.ActivationFunctionType
ALU = mybir.AluOpType
AX = mybir.AxisListType

B, S, DM = 2, 2048, 1024
H, D = 16, 64
NC = 8
ROWS = S * B // NC          # 512 query rows per core
SB = S


# ---- TileContext tail-drain patch: this walrus rejects >1 sync wait on
# CTRL-class instructions; split the global-clock waits onto SP nops.
def _patched_drain_and_barrier(self, tick_clock, wait_clock):
    nc = self.nc
    drain_inst = nc.sync.drain()
    wait_clock.add_sem_waits(
        drain_inst.ins, tile.ScopedClock({None: tick_clock.global_clock})
    )
    si = drain_inst.ins.sync_info
    waits = list(si.on_wait) if si and si.on_wait else []
    if len(waits) > 1:
        si.on_wait = waits[:1]
        for w in waits[1:]:
            nop = nc.sync.nop()
            nop.ins.sync_info = mybir.SyncInfo(on_wait=[w], on_update=[])
    nc.all_engine_barrier()
    assert self.sems is not None
    popped = nc._tile_sem_poison_stack.pop()
    assert popped is self._sem_poison
    nc.all_engine_barrier()


tile.TileContext._drain_and_barrier = _patched_drain_and_barrier

_CTRL_CLASSES = ("InstNoOp", "InstDrain", "InstEventSemaphore")


def _split_excess_waits(nc, maxw_compute=1):
    """Walrus (this version) caps sync waits per instruction (1 for
    CTRL-class, ~2 for compute).  Hoist excess waits onto same-engine NoOps
    inserted immediately before the offending instruction."""
    import copy
    proto = nc.sync.nop().ins  # prototype NoOp (appended to current bb; harmless)
    proto_si = proto.sync_info
    if proto_si and proto_si.on_wait:
        proto.sync_info = mybir.SyncInfo(on_wait=[], on_update=[])
    nsplit = 0
    for f in nc.m.functions:
        for b in f.blocks:
            insts = list(b.instructions)
            out = []
            changed = False
            for inst in insts:
                cls = type(inst).__name__
                maxw = 1 if cls in _CTRL_CLASSES else maxw_compute
                si = inst.sync_info
                waits = list(si.on_wait) if si and si.on_wait else []
                if len(waits) > maxw:
                    keep = waits[:maxw]
                    extra = waits[maxw:]
                    si.on_wait = keep
                    for i, w in enumerate(extra):
                        nop = copy.deepcopy(proto)
                        nop.name = f"{inst.name}-wsplit{i}"
                        nop.engine = inst.engine
                        nop.sync_info = mybir.SyncInfo(on_wait=[w],
                                                       on_update=[])
                        out.append(nop)
                        nsplit += 1
                    changed = True
                out.append(inst)
            if changed:
                try:
                    b.instructions = out
                except Exception:
                    b.set_instructions(out)
    return nsplit


def _build_program():
    nc = bass.Bass("TRN2", target_bir_lowering=False, debug=False,
                   num_devices=NC)

    def din(name, shape, dt):
        return nc.dram_tensor(name, list(shape), dt, kind="ExternalInput").ap()

    xT = din("xT", (DM, SB), bf16)
    xTq = din("xTq", (DM, ROWS), bf16)
    xr = din("xr", (ROWS, DM), bf16)         # x rows + bo + bv@Wo (host)
    wq = din("wq", (DM, DM), bf16)
    wk = din("wk", (DM, DM), bf16)
    wv = din("wv", (DM, DM), bf16)
    wo = din("wo", (DM, DM), bf16)
    ccr = din("ccr", (128, ROWS), bf16)
    ssr = din("ssr", (128, ROWS), bf16)
    bqp = din("bqp", (128, 8), f32)
    bkp = din("bkp", (128, 8), f32)
    gbc = din("gbc", (128, DM), bf16)
    bbc = din("bbc", (128, DM), bf16)
    out = nc.dram_tensor("out", [ROWS, DM], f32, kind="ExternalOutput").ap()
    rs_dram = [nc.dram_tensor(f"rs_stage{g}", [1, (14 if g == 0 else 2) * 512],
                              bf16, kind="Internal").ap() for g in range(2)]
    rinv_dram = [nc.dram_tensor(f"rinv_stage{g}", [14 if g == 0 else 2, 512],
                                bf16, kind="Internal").ap() for g in range(2)]

    RG = [[0, 1, 2, 3], [4, 5, 6, 7]]

    with tile.TileContext(nc) as tc:
        with tc.tile_pool(name="res", bufs=1) as res, \
             tc.tile_pool(name="tmp", bufs=4) as tmp, \
             tc.tile_pool(name="ppool", bufs=8) as ppool, \
             tc.tile_pool(name="dram", bufs=1, space="DRAM") as dpool:

            xq_sb = [res.tile([128, ROWS], bf16, name=f"xq{k}", tag=f"xq{k}") for k in range(8)]
            kT = [res.tile([128, SB], bf16, name=f"kT{t}", tag=f"kT{t}") for t in range(8)]
            qT = [res.tile([128, ROWS], bf16, name=f"qT{t}", tag=f"qT{t}") for t in range(8)]
            vt = [res.tile([128, H * (D + 1)], bf16, name=f"vt{m}", tag=f"vt{m}")
                  for m in range(16)]
            aT = [res.tile([128, ROWS], bf16, name=f"aT{t}", tag=f"aT{t}") for t in range(8)]
            ccr_sb = res.tile([128, ROWS], bf16, tag="ccr")
            ssr_sb = res.tile([128, ROWS], bf16, tag="ssr")
            bq_sb = res.tile([128, 8], f32, tag="bq")
            bk_sb = res.tile([128, 8], f32, tag="bk")
            eps_sb = res.tile([128, 1], f32, tag="eps")

            # load order: K-proj inputs first (wk, xq, rope tables,
            # bias), then wq, wv, and the V-proj x^T tiles last (V MMs
            # start only after K+Q drain anyway).
            for k in range(8):
                nc.sync.dma_start(xq_sb[k][:], xTq[k * 128:(k + 1) * 128, :])
            nc.sync.dma_start(ccr_sb[:], ccr[:])
            nc.sync.dma_start(ssr_sb[:], ssr[:])
            nc.sync.dma_start(bk_sb[:], bkp[:])
            nc.sync.dma_start(bq_sb[:], bqp[:])
            nc.vector.memset(eps_sb[:], 1e-5)

            def rope(dst, src, cct, sst, n0, nn):
                # dst[:, n0:n0+nn] = src*CC + swap32(src)*SS
                # (cross-partition 2-input DVE ops are illegal -> copy first)
                t1 = tmp.tile([128, nn], bf16, tag="ropet1")
                t2 = tmp.tile([128, nn], bf16, tag="ropet2")
                for a, b_ in ((0, 32), (32, 0), (64, 96), (96, 64)):
                    nc.vector.tensor_copy(t2[a:a + 32, :], src[b_:b_ + 32, :])
                nc.vector.tensor_tensor(out=t1[:], in0=src[:],
                                        in1=cct[:, n0:n0 + nn], op=ALU.mult)
                nc.vector.tensor_tensor(out=t2[:], in0=t2[:],
                                        in1=sst[:, n0:n0 + nn], op=ALU.mult)
                nc.vector.tensor_tensor(out=dst[:, n0:n0 + nn], in0=t1[:],
                                        in1=t2[:], op=ALU.add)

            # ---- projections ----
            # Each core projects K only for its OWN 512 positions; four
            # 4-way AllGathers (one per head-pair tile pair, per batch
            # replica group) exchange the RoPEd K^T blocks while the PE
            # does Q and the (redundant) V projection.
            with tc.tile_pool(name="wts", bufs=1) as wts, \
                 tc.tile_pool(name="psP", bufs=5, space="PSUM") as psP:
                kin_q = [dpool.tile([256, 512], bf16, name=f"kin_{g}")
                         for g in range(4)]
                kout_q = [dpool.tile([1024, 512], bf16, name=f"kout_{g}")
                          for g in range(4)]

                wk_sb = [wts.tile([128, DM], bf16, name=f"wk{k}", tag=f"wk{k}")
                         for k in range(8)]
                wv_sb = [wts.tile([128, DM], bf16, name=f"wv{k}", tag=f"wv{k}")
                         for k in range(8)]
                xt_sb = [wts.tile([128, SB], bf16, name=f"xt{k}",
                                  tag=f"xt{k}") for k in range(8)]
                kT_own = [wts.tile([128, 512], bf16, name=f"ko{t}",
                                   tag=f"ko{t}") for t in range(8)]
                for k in range(8):
                    nc.sync.dma_start(wk_sb[k][:], wk[k * 128:(k + 1) * 128, :])

                # K^T projection (own 512 positions) + RoPE, then AllGather
                cc_k = [None] * 4
                kin_dmas = []
                for t in range(8):
                    ps = psP.tile([128, 512], f32, tag="proj")
                    for kd in range(8):
                        nc.tensor.matmul(
                            ps[:], wk_sb[kd][:, t * 128:(t + 1) * 128],
                            xq_sb[kd][:], start=(kd == 0), stop=(kd == 7))
                    kt_raw = tmp.tile([128, 512], bf16, tag="evac")
                    nc.scalar.activation(kt_raw[:], ps[:], AF.Identity,
                                         bias=bk_sb[:, t:t + 1])
                    rope(kT_own[t], kt_raw, ccr_sb, ssr_sb, 0, 512)
                    g, tt = t // 2, t % 2
                    # staged from the scalar queue: a sync-queue DMA here
                    # would head-of-line-block the wq/xT/wv loads behind it
                    # while waiting on the RoPE.
                    dma = nc.scalar.dma_start(
                        kin_q[g][tt * 128:(tt + 1) * 128, :], kT_own[t][:])
                    kin_dmas.append(dma)
                    if tt == 1:
                        cc = nc.gpsimd.collective_compute(
                            "AllGather", ALU.bypass, replica_groups=RG,
                            ins=[kin_q[g].opt()], outs=[kout_q[g].opt()])
                        # DRAM tiles are not dependency-tracked: tie the
                        # trigger to the two staging DMAs explicitly.
                        for d_ in kin_dmas[-2:]:
                            bass._add_dep_helper(cc.ins, d_.ins, sync=True,
                                                 reason="AG_K input staged")
                        cc_k[g] = cc

                # wv + x^T stream right behind wk; wq last (Q-proj runs
                # AFTER the V projection, filling the PE gap between
                # V-drain and the first exp).
                for k in range(8):
                    nc.sync.dma_start(wv_sb[k][:], wv[k * 128:(k + 1) * 128, :])
                for k in range(8):
                    nc.sync.dma_start(xt_sb[k][:], xT[k * 128:(k + 1) * 128, :])
                wq_sb = [wts.tile([128, DM], bf16, name=f"wq{k}", tag=f"wk{k}")
                         for k in range(8)]
                for k in range(8):
                    nc.sync.dma_start(wq_sb[k][:], wq[k * 128:(k + 1) * 128, :])

                # V projection (redundant, all 2048 positions; 65-stride
                # head slots + ones column for the softmax rowsums).
                # Scalar-engine evac: Act is idle until the first Exp.
                for m in range(16):
                    m0 = m * 128
                    pss = [psP.tile([128, 512], f32, tag="proj",
                                    name=f"vps{m}_{ncol}")
                           for ncol in range(2)]
                    # kd-inner: consecutive MM pairs share the stationary
                    # xt slice, so the weight load amortizes over 1024
                    # streamed columns.
                    for kd in range(8):
                        for ncol in range(2):
                            nc.tensor.matmul(
                                pss[ncol][:], xt_sb[kd][:, m0:m0 + 128],
                                wv_sb[kd][:, ncol * 512:ncol * 512 + 512],
                                start=(kd == 0), stop=(kd == 7))
                    for ncol in range(2):
                        dst = vt[m][:, ncol * 8 * 65:(ncol + 1) * 8 * 65]
                        dstv = dst.rearrange("p (h e) -> p h e", e=65)[:, :, 0:64]
                        srcv = pss[ncol][:].rearrange("p (h e) -> p h e", e=64)
                        nc.scalar.activation(dstv, srcv, AF.Identity)
                    onev = vt[m][:, :].rearrange("p (h e) -> p h e",
                                                 e=65)[:, :, 64:65]
                    nc.vector.memset(onev, 1.0)

                # Q^T projection + RoPE, after V: the first exp needs
                # qT[0] only once the V drain + first scores are done.
                for t in range(8):
                    ps = psP.tile([128, 512], f32, tag="proj")
                    for kd in range(8):
                        nc.tensor.matmul(
                            ps[:], wq_sb[kd][:, t * 128:(t + 1) * 128],
                            xq_sb[kd][:], start=(kd == 0), stop=(kd == 7))
                    q_raw = tmp.tile([128, ROWS], bf16, tag="evac")
                    nc.scalar.activation(q_raw[:], ps[:], AF.Identity,
                                         bias=bq_sb[:, t:t + 1])
                    rope(qT[t], q_raw, ccr_sb, ssr_sb, 0, ROWS)

                # gathered K^T -> attention layout (sync queue, after all
                # critical loads; explicit dep edges onto the collectives).
                for g in range(4):
                    for i in range(4):
                        for tt in range(2):
                            t = g * 2 + tt
                            dma = nc.sync.dma_start(
                                kT[t][:, i * 512:(i + 1) * 512],
                                kout_q[g][i * 256 + tt * 128:
                                          i * 256 + (tt + 1) * 128, :])
                            bass._add_dep_helper(dma.ins, cc_k[g].ins,
                                                 sync=True,
                                                 reason="AG_K output read")

            # ---- attention ----
            # scores transposed (S^T = K^T-chunk @ Q^T) into [128,1024] PSUM
            # mega-tiles so each Exp covers FD=1024.  The two heads of a
            # t-pair are emitted alternating so their score MMs stream
            # concurrently through PE row groups 0-63 / 64-127.  PV lags one
            # kc-pair.  Rowsums ride the V ones-column; normalization is
            # deferred and applied to the bf16 aT tiles -- heads 0-11
            # mid-attention (after t=5), heads 12-15 at the tail.
            with tc.tile_pool(name="asb", bufs=1) as asb:
                # rowsum rows stage into one SBUF row; a DRAM bounce
                # scatters them across partitions so ONE batched DVE
                # reciprocal runs at FD=512 (DVE reciprocal is iterative,
                # ~6.5ns/elem along the free dim -- partition-packing is
                # what makes it cheap).  rinvA holds the scatter back at
                # partitions {0,32}: head 2g+i -> partition 32i, col g*512.
                rs_row = asb.tile([65, H * 512], bf16, tag="rs_row")
                rsS = [asb.tile([14, 512], bf16, tag="rsS0", name="rsS0")]
                rinvS = [asb.tile([14, 512], bf16, tag="rinvS0",
                                  name="rinvS0")]
                rsSb = asb.tile([33, 512], bf16, tag="rsSb")
                rinvSb = asb.tile([33, 512], bf16, tag="rinvSb")
                rinvA = asb.tile([33, 7 * 512], bf16, tag="rinvA")
                onesA = asb.tile([33, 64], bf16, tag="onesA")
                nc.vector.memset(onesA[:], 1.0)

                with tc.tile_pool(name="psA", bufs=3, space="PSUM") as psA, \
                     tc.tile_pool(name="psO", bufs=2, space="PSUM") as psO:

                    prev_exp_h0 = [None]
                    for t in range(8):
                        oaccs = [psO.tile([65, 512], f32, tag="oacc",
                                          name=f"oacc{t}_{hh}")
                                 for hh in range(2)]
                        prev = [None, None]

                        def emit_pv(hh, kp, pT_t):
                            h = 2 * t + hh
                            for j in range(2):
                                kc = kp * 2 + j
                                nc.tensor.matmul(
                                    oaccs[hh][:],
                                    vt[kc][:, h * 65:h * 65 + 65],
                                    pT_t[:, j * 512:(j + 1) * 512],
                                    start=(kc == 0), stop=(kc == 15))

                        for kp in range(8):
                            sps = [psA.tile([128, 1024], f32, tag="sco",
                                            name=f"sco{t}_{kp}_{hh}")
                                   for hh in range(2)]
                            # hh-alternated score MMs: row groups 0-63 and
                            # 64-127 stream concurrently.
                            for j in range(2):
                                kc = kp * 2 + j
                                for hh in range(2):
                                    po = 64 * hh
                                    mm = nc.tensor.matmul(
                                        sps[hh][:, j * 512:(j + 1) * 512],
                                        kT[t][po:po + 64,
                                              kc * 128:(kc + 1) * 128],
                                        qT[t][po:po + 64, :],
                                        start=True, stop=True)
                                    # equalize readiness on the EARLIER of
                                    # the previous kp's exps: all 4 score
                                    # MMs become ready together, so the
                                    # scheduler issues them adjacently and
                                    # the h0/h64 row-group pairs can
                                    # stream concurrently.
                                    if prev_exp_h0[0] is not None:
                                        bass._add_dep_helper(
                                            mm.ins, prev_exp_h0[0].ins,
                                            sync=True,
                                            reason="score pair readiness")
                            for hh in range(2):
                                pT = ppool.tile([128, 1024], bf16, tag="pT")
                                ex = nc.scalar.activation(pT[:], sps[hh][:],
                                                          AF.Exp, scale=0.125)
                                if hh == 0:
                                    prev_exp_h0[0] = ex
                                if prev[hh] is not None:
                                    emit_pv(hh, kp - 1, prev[hh])
                                prev[hh] = pT
                        for hh in range(2):
                            emit_pv(hh, 7, prev[hh])
                        # stash rowsum rows (same-partition copies) +
                        # unnorm. attn (out-partition shift legal for
                        # 1-input copies)
                        for hh in range(2):
                            h, po = 2 * t + hh, 64 * hh
                            nc.vector.tensor_copy(
                                rs_row[64:65, h * 512:(h + 1) * 512],
                                oaccs[hh][64:65, :])
                            nc.vector.tensor_copy(aT[t][po:po + 64, :],
                                                  oaccs[hh][0:64, :])
                        # reciprocal batches: heads 0-13 after t=6 (the
                        # bounce + recip hide inside t=7's exp window);
                        # heads 14-15 at the tail via a SHORT chain: the
                        # bounce-back lands rows directly on partitions
                        # {0,32} so the broadcast MM reads the reciprocal
                        # with no second scatter hop.
                        if t == 6:
                            nc.sync.dma_start(rs_dram[0][:],
                                              rs_row[64:65, 0:14 * 512])
                            nc.sync.dma_start(
                                rsS[0][:],
                                rs_dram[0].rearrange("a (p c) -> (a p) c",
                                                     p=14))
                            with nc.allow_low_precision(
                                    reason="softmax 1/rowsum in bf16"):
                                nc.vector.reciprocal(rinvS[0][:], rsS[0][:])
                            nc.sync.dma_start(rinv_dram[0][:], rinvS[0][:])
                            for i in range(2):
                                nc.sync.dma_start(
                                    rinvA[32 * i:32 * i + 1,
                                          0:7 * 512].rearrange(
                                        "a (g c) -> a g c", c=512),
                                    rinv_dram[0].rearrange(
                                        "(g i) c -> i g c", i=2)[i:i + 1])
                        if t == 7:
                            nc.sync.dma_start(rs_dram[1][:],
                                              rs_row[64:65, 14 * 512:16 * 512])
                            for i in range(2):
                                nc.sync.dma_start(
                                    rsSb[32 * i:32 * i + 1, :],
                                    rs_dram[1][:, i * 512:(i + 1) * 512])
                            with nc.allow_low_precision(
                                    reason="softmax 1/rowsum in bf16"):
                                nc.vector.reciprocal(rinvSb[:], rsSb[:])
                # ---- out-proj + residual + LayerNorm ----
                # (psA/psO closed -> PSUM free for psF + psB.)
                # All 16 head-normalizes run here, interleaved with the
                # out-proj kd accumulation so the PE never waits: head
                # pair 2t,2t+1 normalizes, then kd=t accumulates.
                with tc.tile_pool(name="wop", bufs=1) as wop, \
                     tc.tile_pool(name="fin", bufs=2) as fin, \
                     tc.tile_pool(name="psB", bufs=4, space="PSUM") as psB, \
                     tc.tile_pool(name="psF", bufs=4, space="PSUM") as psF:

                    def normalize_pair(t):
                        # both heads' 1/rowsum broadcasts via col-tiled PE
                        # outer products into ONE PSUM tile; Act (idle
                        # post-Exp, Identity stays in the exp table set)
                        # evacuates so the DVE multiply runs in 2x bf16
                        # mode as a single [128,512] op.
                        rt, rc = (rinvA, t * 512) if t < 7 else (rinvSb, 0)
                        bc = psB.tile([128, 512], f32, tag="bc")
                        for hh in range(2):
                            po = 64 * hh
                            nc.tensor.matmul(
                                bc[po:po + 64, :],
                                onesA[32 * hh:32 * hh + 1, :],
                                rt[32 * hh:32 * hh + 1, rc:rc + 512],
                                start=True, stop=True)
                        bcs = tmp.tile([128, 512], bf16, tag="bcs")
                        nc.scalar.activation(bcs[:], bc[:], AF.Identity)
                        nc.vector.tensor_tensor(out=aT[t][:], in0=aT[t][:],
                                                in1=bcs[:], op=ALU.mult)
                    wo_sb = [wop.tile([128, DM], bf16, name=f"wo{k}",
                                      tag=f"wo{k}") for k in range(8)]
                    for k in range(8):
                        nc.sync.dma_start(wo_sb[k][:],
                                          wo[k * 128:(k + 1) * 128, :])
                    g_sb = wop.tile([128, DM], bf16, tag="g")
                    b_sb = wop.tile([128, DM], bf16, tag="b")
                    nc.sync.dma_start(g_sb[:], gbc[:])
                    nc.sync.dma_start(b_sb[:], bbc[:])

                    def outproj(psf, mrs, kds):
                        for kd in kds:
                            for mr in mrs:
                                rr = mr * 128
                                for ncol in range(2):
                                    c0 = ncol * 512
                                    nc.tensor.matmul(
                                        psf[(mr, ncol)][:],
                                        aT[kd][:, rr:rr + 128],
                                        wo_sb[kd][:, c0:c0 + 512],
                                        start=(kd == 0), stop=(kd == 7))

                    def ln_rows(psf, mr):
                        rr = mr * 128
                        xb = fin.tile([128, DM], bf16, tag="xb")
                        nc.sync.dma_start(xb[:], xr[rr:rr + 128, :])
                        # LN: one bn_stats pass for mean+var; the (h-mu)*
                        # rstd affine runs on the (post-Exp idle) Act
                        # engine as a single Identity with per-partition
                        # scale/bias.  bf16 intermediates: abs error ~2e-3
                        # of a ~5.0-max output, well inside tolerance.
                        hrow = fin.tile([128, DM], bf16, tag="hrow")
                        for ncol in range(2):
                            c0 = ncol * 512
                            nc.vector.tensor_tensor(
                                out=hrow[:, c0:c0 + 512],
                                in0=psf[(mr, ncol)][:],
                                in1=xb[:, c0:c0 + 512], op=ALU.add)
                        stats = fin.tile([128, 12], f32, tag="stats")
                        for g in range(2):
                            nc.vector.bn_stats(
                                stats[:, 6 * g:6 * g + 6],
                                hrow[:, 512 * g:512 * g + 512])
                        mv = fin.tile([128, 2], f32, tag="mv")
                        nc.vector.bn_aggr(mv[:], stats[:])
                        st = fin.tile([128, 1], f32, tag="st")
                        nc.scalar.activation(st[:], mv[:, 1:2], AF.Sqrt,
                                             bias=eps_sb[:])
                        rstd = fin.tile([128, 1], f32, tag="rstd")
                        nc.vector.reciprocal(rstd[:], st[:])
                        nmr = fin.tile([128, 1], f32, tag="nmr")
                        nc.vector.tensor_scalar(out=nmr[:], in0=mv[:, 0:1],
                                                scalar1=rstd[:],
                                                scalar2=-1.0, op0=ALU.mult,
                                                op1=ALU.mult)
                        y = fin.tile([128, DM], bf16, tag="y")
                        nc.scalar.activation(y[:], hrow[:], AF.Identity,
                                             bias=nmr[:], scale=rstd[:])
                        nc.vector.tensor_tensor(out=y[:], in0=y[:],
                                                in1=g_sb[:], op=ALU.mult)
                        yf = fin.tile([128, DM], f32, tag="yf")
                        nc.vector.tensor_tensor(out=yf[:], in0=y[:],
                                                in1=b_sb[:], op=ALU.add)
                        nc.sync.dma_start(out[rr:rr + 128, :], yf[:])

                    # 4 psF tags x 1 buf = 4 banks, two mr-rows in
                    # flight; later quarters reuse earlier slots (the
                    # LN PSUM reads are emitted first, so the slot-reuse
                    # anti-dependency edges are recorded).
                    for t in range(7):
                        normalize_pair(t)
                    psfq = []
                    for mr in range(4):
                        psf = {(mr, ncol): psF.tile(
                            [128, 512], f32, bufs=1,
                            tag=f"f{mr % 2}_{ncol}",
                            name=f"psf{mr}_{ncol}")
                            for ncol in range(2)}
                        outproj(psf, (mr,), range(7))
                        if mr == 0:
                            normalize_pair(7)
                        outproj(psf, (mr,), (7,))
                        ln_rows(psf, mr)

    _split_excess_waits(nc)
    return nc


_NC_CACHE = None


def _perm():
    p = np.zeros(DM, np.int64)
    for h in range(H):
        p[h * D:h * D + 32] = h * D + np.arange(0, D, 2)
        p[h * D + 32:(h + 1) * D] = h * D + np.arange(1, D, 2)
    return p


def kernel(x, Wqkv, bqkv, Wo, bo, gamma, beta):
    global _NC_CACHE
    x = np.asarray(x, np.float32)
    Wqkv = np.asarray(Wqkv, np.float32)
    bqkv = np.asarray(bqkv, np.float32)
    Wo = np.asarray(Wo, np.float32)
    bo = np.asarray(bo, np.float32)
    gamma = np.asarray(gamma, np.float32)
    beta = np.asarray(beta, np.float32)

    perm = _perm()
    Wq = np.ascontiguousarray(Wqkv[:, 0:DM][:, perm]).astype(BF16)
    Wk = np.ascontiguousarray(Wqkv[:, DM:2 * DM][:, perm]).astype(BF16)
    Wv = np.ascontiguousarray(Wqkv[:, 2 * DM:3 * DM]).astype(BF16)
    Wob = Wo.astype(BF16)
    bq = bqkv[0:DM][perm]
    bk = bqkv[DM:2 * DM][perm]
    bv = bqkv[2 * DM:3 * DM]

    inv = 1.0 / (10000.0 ** (np.arange(0, D, 2, dtype=np.float64) / D))
    pos = np.arange(S, dtype=np.float64)
    fr = pos[None, :] * inv[:, None]                    # [32, S]
    c32, s32 = np.cos(fr), np.sin(fr)
    CC = np.concatenate([c32, c32, c32, c32], 0).astype(BF16)   # [128, S]
    SS = np.concatenate([-s32, s32, -s32, s32], 0).astype(BF16)

    def colmajor(v):
        return np.ascontiguousarray(v.reshape(8, 128).T).astype(np.float32)

    gB = np.ascontiguousarray(np.broadcast_to(gamma, (128, DM))).astype(BF16)
    bB = np.ascontiguousarray(np.broadcast_to(beta, (128, DM))).astype(BF16)
    # bv contributes bv @ Wo to every output row (softmax weights sum to 1);
    # fold it, with bo, into the residual rows on host.
    rbias = bo + bv @ Wo

    if _NC_CACHE is None:
        _NC_CACHE = _build_program()
    nc = _NC_CACHE

    in_maps = []
    for c in range(NC):
        b, r = c // 4, c % 4
        xTb = np.ascontiguousarray(x[b].T).astype(BF16)
        rr = r * ROWS
        in_maps.append({
            "xT": xTb,
            "xTq": np.ascontiguousarray(xTb[:, rr:rr + ROWS]),
            "xr": np.ascontiguousarray(
                x[b, rr:rr + ROWS, :] + rbias[None, :]).astype(BF16),
            "wq": Wq, "wk": Wk, "wv": Wv, "wo": Wob,
            "ccr": np.ascontiguousarray(CC[:, rr:rr + ROWS]),
            "ssr": np.ascontiguousarray(SS[:, rr:rr + ROWS]),
            "bqp": colmajor(bq), "bkp": colmajor(bk),
            "gbc": gB, "bbc": bB,
        })

    res = run_bass_kernel_spmd(nc, in_maps, core_ids=list(range(NC)))
    kernel._last_results = res
    full = np.empty((B, S, DM), np.float32)
    for c in range(NC):
        b, r = c // 4, c % 4
        full[b, r * ROWS:(r + 1) * ROWS, :] = res.results[c]["out"]
    return full
